# revision 2
# baseline (speedup 1.0000x reference)
"""Trainium2 Bass kernel for nn_ClassLayer_56564719289025 — fp8 DoubleRow version.

Reference computation:  y = mean(|W|) * (x @ sign(W).T)
  x: [8192, 4096] f32, W: [4096, 4096] f32 -> y: [8192, 4096] f32

Strategy (8 NeuronCores):
  - Data-parallel over x rows: each core computes a 1024-row shard of y.
  - sign(W) is exactly representable in fp8e4 (e4m3); x is shipped as a
    two-term e4m3 expansion x ~= a + b (a = e4m3(x), b = e4m3(x - a),
    rel err ~7.5e-4), so the matmul runs in fp8 DoubleRow perf mode
    (256-deep contraction per pass at ~2.2x the bf16 pass rate, measured
    90.6ns per [128x256]@[256x512] pass vs 206ns for bf16 [128x128]@[128x512]).
  - Each PSUM tile accumulates 2 rounds (a, b) x 16 DoubleRow k-pairs = 32
    matmuls; chains are closed per tile (start/stop), which is the fast
    accumulation regime on hardware.
  - W is replicated in bf16; ACT converts each W tile to a ±1 fp8 sign tile.
  - scale = mean(|W|) on-device: per-core abs-sum of a 512-column slice of
    W^T, then a [128]x f32 AllReduce across the 8 cores (as in the bf16
    baseline kernel).

Per-core loop: a^T/b^T shards resident in SBUF (fp8, 32KB/partition each);
W^T streamed in 8 o-blocks of 512 columns; PE runs 2048 DoubleRow matmuls;
DVE evicts PSUM with a deferred multiply by the broadcast scale.
"""

import numpy as np
import ml_dtypes

import concourse.bacc as bacc
import concourse.bass_isa as bass_isa
import concourse.mybir as mybir
import concourse.tile as tile
from concourse.bass_utils import run_bass_kernel_spmd

TOKENS, D_IN, D_OUT, N_CORES = 8192, 4096, 4096, 8
P = 128            # SBUF partitions / matmul k-subtile
OB = 512           # output-column block (one PSUM bank at fp32)
R_SHARD = TOKENS // N_CORES   # 1024 rows per core
KO = D_IN // P                # 32 contraction subtiles
KP = KO // 2                  # 16 DoubleRow k-pairs
NB = D_OUT // OB              # 8 o-blocks
RT = R_SHARD // P             # 8 row tiles per core
SCAN_W = D_OUT // N_CORES     # 512-column scan slice per core
INV_N = 1.0 / (D_IN * D_OUT)  # exactly 2**-24

bf16 = mybir.dt.bfloat16
fp8 = mybir.dt.float8e4
fp32 = mybir.dt.float32
DR = mybir.MatmulPerfMode.DoubleRow


def _emit(tc, aT, bT, wT, wS, y, part, red, reps=1):
    nc = tc.nc
    aT3 = aT.rearrange("(ko p) r -> p ko r", p=P)   # [128, 32, 1024]
    bT3 = bT.rearrange("(ko p) r -> p ko r", p=P)   # [128, 32, 1024]
    wT3 = wT.rearrange("(ko p) o -> p ko o", p=P)   # [128, 32, 4096]
    wS3 = wS.rearrange("(ko p) o -> p ko o", p=P)   # [128, 32, 512]
    y3 = y.rearrange("(rt p) o -> p rt o", p=P)     # [128, 8, 4096]

    with (
        tc.tile_pool(name="xpool", bufs=1) as xpool,
        tc.tile_pool(name="spool", bufs=2) as spool,
        tc.tile_pool(name="wmain", bufs=8) as wmain,
        tc.tile_pool(name="wscan", bufs=2) as wscan,
        tc.tile_pool(name="scpool", bufs=1) as scpool,
        tc.tile_pool(name="ypool", bufs=12) as ypool,
        tc.tile_pool(name="psum", bufs=8, space="PSUM") as psum,
    ):
        for _ in range(reps):
            def produce_S(b):
                # 8-k-subtile chunks: one DMA + one ACT op per chunk amortizes
                # the ~352-cycle ACT instruction overhead
                S_b = spool.tile([P, KO, OB], fp8, tag="S")
                for g in range(KO // 8):
                    w_t = wmain.tile([P, 8, OB], bf16, tag="w")
                    nc.sync.dma_start(
                        w_t[:], wT3[:, g * 8:(g + 1) * 8, b * OB:(b + 1) * OB]
                    )
                    nc.scalar.sign(S_b[:, g * 8:(g + 1) * 8, :], w_t[:])
                return S_b

            # --- a/b shards and block-0 W loaded k-sliced, interleaved so the
            # k-outer block-0 matmuls can start after one (a[pair], S0[pair]) ---
            a_sb = xpool.tile([P, KO, R_SHARD], fp8, tag="a")
            b_sb = xpool.tile([P, KO, R_SHARD], fp8, tag="b")
            S0 = spool.tile([P, KO, OB], fp8, tag="S")
            for g in range(KO // 8):
                w_t = wmain.tile([P, 8, OB], bf16, tag="w")
                nc.sync.dma_start(w_t[:], wT3[:, g * 8:(g + 1) * 8, 0:OB])
                nc.scalar.sign(S0[:, g * 8:(g + 1) * 8, :], w_t[:])
                for k in range(g * 8, (g + 1) * 8):
                    nc.sync.dma_start(a_sb[:, k, :], aT3[:, k, :])
            for k in range(KO):
                nc.sync.dma_start(b_sb[:, k, :], bT3[:, k, :])

            acc = scpool.tile([P, KO], fp32, tag="acc")

            def scan_j(j):
                ws_t = wscan.tile([P, 8, OB], bf16, tag="ws")
                nc.sync.dma_start(ws_t[:], wS3[:, j * 8:(j + 1) * 8, :])
                nc.vector.tensor_reduce(
                    acc[:, j * 8:(j + 1) * 8], ws_t[:],
                    axis=mybir.AxisListType.X, op=mybir.AluOpType.add,
                    apply_absolute_value=True,
                )

            for j in range(4):
                scan_j(j)

            # --- finish scale: partition reduce, cross-core AllReduce ---
            acc1 = scpool.tile([P, 1], fp32, tag="acc1")
            nc.vector.tensor_reduce(
                acc1[:], acc[:], axis=mybir.AxisListType.X, op=mybir.AluOpType.add
            )
            accs = scpool.tile([P, 1], fp32, tag="accs")
            nc.vector.tensor_scalar_mul(accs[:], acc1[:], INV_N)
            par_t = scpool.tile([P, 1], fp32, tag="par")
            nc.gpsimd.partition_all_reduce(
                par_t[:], accs[:], channels=P, reduce_op=bass_isa.ReduceOp.add
            )
            nc.sync.dma_start(part[:], par_t[:])
            nc.gpsimd.collective_compute(
                "AllReduce", mybir.AluOpType.add,
                [list(range(N_CORES))], [part[:]], [red[:]],
            )
            scale_sb = scpool.tile([P, 1], fp32, tag="scale")
            nc.sync.dma_start(scale_sb[:], red[:])

            # --- block 1 prefetch ---
            S1 = produce_S(1)

            def evict(ps, r, b):
                # two-step eviction: DVE copy frees the PSUM bank without
                # waiting on scale; the scale multiply binds later
                y_t = ypool.tile([P, OB], fp32, tag="y")
                nc.vector.tensor_copy(out=y_t[:], in_=ps[:])
                nc.vector.tensor_scalar_mul(y_t[:], y_t[:], scale_sb[:])
                nc.sync.dma_start(y3[:, r, b * OB:(b + 1) * OB], y_t[:])

            def mm(ps, src3, S_b, kk, r, start, stop):
                nc.tensor.matmul(
                    ps[:],
                    lhsT=src3[:, 2 * kk:2 * kk + 2, r * P:(r + 1) * P],
                    rhs=S_b[:, 2 * kk:2 * kk + 2, :],
                    start=start,
                    stop=stop,
                    perf_mode=DR,
                )

            # --- block 0: k-outer over 8 concurrent PSUM banks, so the PE
            # starts on the first (a[pair], S0[pair]) and tracks DMA supply ---
            ps0 = [
                psum.tile([P, OB], fp32, tag="ps", name=f"ps0_{r}")
                for r in range(RT)
            ]
            for kk in range(KP):
                for r in range(RT):
                    mm(ps0[r], a_sb, S0, kk, r, start=(kk == 0), stop=False)
            for kk in range(KP):
                for r in range(RT):
                    mm(ps0[r], b_sb, S0, kk, r, start=False, stop=(kk == KP - 1))
            for r in range(RT):
                evict(ps0[r], r, 0)

            # --- blocks 1..7: r-outer, 32-pass closed chain per PSUM tile ---
            for b in range(1, NB):
                S_b = S1 if b == 1 else produce_S(b)
                for r in range(RT):
                    ps = psum.tile([P, OB], fp32, tag="ps")
                    for kk in range(KP):
                        mm(ps, a_sb, S_b, kk, r, start=(kk == 0), stop=False)
                    for kk in range(KP):
                        mm(ps, b_sb, S_b, kk, r, start=False, stop=(kk == KP - 1))
                    evict(ps, r, b)


def build(reps=1):
    nc = bacc.Bacc(
        "TRN2", target_bir_lowering=False, debug=False, num_devices=N_CORES
    )
    aT = nc.dram_tensor("aT", [D_IN, R_SHARD], fp8, kind="ExternalInput").ap()
    bT = nc.dram_tensor("bT", [D_IN, R_SHARD], fp8, kind="ExternalInput").ap()
    wT = nc.dram_tensor("wT", [D_IN, D_OUT], bf16, kind="ExternalInput").ap()
    wS = nc.dram_tensor("wscan", [D_IN, SCAN_W], bf16, kind="ExternalInput").ap()
    y = nc.dram_tensor("y", [R_SHARD, D_OUT], fp32, kind="ExternalOutput").ap()
    part = nc.dram_tensor("part", [P, 1], fp32, kind="Internal").ap()
    red = nc.dram_tensor("red", [P, 1], fp32, kind="Internal", addr_space="Shared").ap()

    with tile.TileContext(nc) as tc:
        _emit(tc, aT, bT, wT, wS, y, part, red, reps=reps)
    nc.compile()
    return nc


_NC_CACHE = {}


def _get_nc(reps=1):
    if reps not in _NC_CACHE:
        _NC_CACHE[reps] = build(reps)
    return _NC_CACHE[reps]


def _make_in_maps(x, weight):
    E4 = ml_dtypes.float8_e4m3
    xf = np.asarray(x, dtype=np.float32)
    a = xf.astype(E4)
    b = (xf - a.astype(np.float32)).astype(E4)
    wb = np.asarray(weight).astype(ml_dtypes.bfloat16)
    aTb = np.ascontiguousarray(a.T)    # [D_IN, TOKENS] fp8
    bTb = np.ascontiguousarray(b.T)    # [D_IN, TOKENS] fp8
    wTb = np.ascontiguousarray(wb.T)   # [D_IN, D_OUT] bf16
    in_maps = []
    for c in range(N_CORES):
        in_maps.append({
            "aT": np.ascontiguousarray(aTb[:, c * R_SHARD:(c + 1) * R_SHARD]),
            "bT": np.ascontiguousarray(bTb[:, c * R_SHARD:(c + 1) * R_SHARD]),
            "wT": wTb,
            "wscan": np.ascontiguousarray(wTb[:, c * SCAN_W:(c + 1) * SCAN_W]),
        })
    return in_maps


def kernel(x, weight):
    x = np.asarray(x)
    weight = np.asarray(weight)
    assert x.shape == (TOKENS, D_IN), x.shape
    assert weight.shape == (D_OUT, D_IN), weight.shape
    in_maps = _make_in_maps(x, weight)
    nc = _get_nc(1)
    last_exc = None
    for attempt in range(3):
        try:
            res = run_bass_kernel_spmd(nc, in_maps, core_ids=list(range(N_CORES)))
            break
        except Exception as e:  # transient NRT device errors — retry
            last_exc = e
            import time as _time

            _time.sleep(2.0 * (attempt + 1))
    else:
        raise last_exc
    return np.concatenate(
        [res.results[c]["y"] for c in range(N_CORES)], axis=0
    ).astype(np.float32)
